# revision 3
# baseline (speedup 1.0000x reference)
"""Self-contained Trainium2 Bass kernel for the HQNN problem.

Math: the 4-qubit circuit after angle embedding applies a fixed unitary whose
Heisenberg-evolved Z observables are sparse Pauli sums over {I,Y,Z}; each
hybrid layer reduces to tanh -> sin/cos -> a few elementwise products -> small
static matmuls (folded with the next Dense layer). Data-parallel over 8 cores.
"""
import os
import sys
sys.path.insert(0, "/opt/trn_rl_repo")
import itertools
import contextlib
import numpy as np

# bench-only: repeat the kernel body N times so on-HW body time can be
# measured as a slope over reps (dispatch overhead dominates single calls).
BENCH_REPS = int(os.environ.get("BASS_BENCH_REPS", "1"))

import concourse.bass as bass
import concourse.bacc as bacc
import concourse.tile as tile
from concourse import mybir
from concourse.bass_utils import run_bass_kernel_spmd
from concourse.masks import make_identity

F32 = mybir.dt.float32
PI2 = float(np.pi / 2)
N_CORES = 8
B_TOTAL, D_IN = 524288, 16
B_CORE = B_TOTAL // N_CORES

# ---------------- host-side math ----------------
_I2 = np.eye(2, dtype=complex)
_PY = np.array([[0, -1j], [1j, 0]])
_PZ = np.array([[1, 0], [0, -1]], dtype=complex)
SUPPORTS = [(0, 1, 3), (0, 2, 3), (1, 3), (0, 2)]


def _kron(ms):
    out = np.array([[1.0 + 0j]])
    for m in ms:
        out = np.kron(out, m)
    return out


def _op_on(w, m):
    return _kron([m if v == w else _I2 for v in range(4)])


def _layer_tensors(theta_l):
    U = np.eye(16, dtype=complex)
    for l in range(2):
        for w in range(4):
            c, s = np.cos(theta_l[l, w] / 2), np.sin(theta_l[l, w] / 2)
            U = _op_on(w, np.array([[c, -1j * s], [-1j * s, c]])) @ U
        for w in range(4):
            t = (w + 1) % 4
            C = np.zeros((16, 16), dtype=complex)
            for k in range(16):
                bits = [(k >> (3 - v)) & 1 for v in range(4)]
                if bits[w] == 1:
                    bits[t] ^= 1
                C[sum(b << (3 - v) for v, b in enumerate(bits)), k] = 1
            U = C @ U
    letters = {"I": _I2, "Y": _PY, "Z": _PZ}
    out = []
    for w, sup in enumerate(SUPPORTS):
        H = U.conj().T @ _op_on(w, _PZ) @ U
        T = np.zeros((2,) * len(sup))
        for s in itertools.product("IYZ", repeat=4):
            P = _kron([letters[c] for c in s])
            co = float(np.real(np.trace(P.conj().T @ H) / 16))
            if abs(co) < 1e-10:
                continue
            nz = tuple(v for v in range(4) if s[v] != "I")
            assert set(nz).issubset(set(sup)), f"support {s} w={w}"
            idx, sign = [], 1.0
            ok = True
            for v in sup:
                if s[v] == "I":
                    ok = False
                    break
                idx.append(0 if s[v] == "Y" else 1)
                if s[v] == "Y":
                    sign = -sign
            if not ok:
                assert abs(co) < 1e-10
                continue
            T[tuple(idx)] = sign * co
        out.append(T)
    return out  # C0, C1, B2, B3


def _blockdiag(blk, n):
    K, M = blk.shape
    out = np.zeros((K * n, M * n), dtype=np.float32)
    for i in range(n):
        out[i * K:(i + 1) * K, i * M:(i + 1) * M] = blk
    return out


WMAP = [3, 0, 1, 2, 3, 0, 1, 2]


def host_tensors(theta, W0, b0, W1, b1, W2, b2):
    t = {}
    for i in range(3):
        C0, C1, B2, B3 = _layer_tensors(np.asarray(theta[i], dtype=np.float64))
        A1 = np.zeros((8, 8), dtype=np.float32)
        for a in range(2):
            for c in range(2):
                gi = a * 2 + c
                A1[gi, 1] = C0[a, 0, c]
                A1[gi, 5] = C0[a, 1, c]
                A1[gi, 2] = C1[a, 0, c]
                A1[gi, 6] = C1[a, 1, c]
        A2 = np.zeros((8, 8), dtype=np.float32)
        for b in range(2):
            A2[1 + 4 * b, 3] = B2[b, 0]
            A2[1 + 4 * b, 7] = B2[b, 1]
            A2[2 + 4 * b, 0] = B3[0, b]
            A2[2 + 4 * b, 4] = B3[1, b]
        t[f"lA1_{i}"] = _blockdiag(A1, 16)
        t[f"lA2_{i}"] = _blockdiag(A2, 16)
    D0 = np.zeros((16, 8), dtype=np.float32)
    D0[:, 0:4] = W0
    D0[:, 4:8] = W0
    t["lD0"] = _blockdiag(D0, 8)
    for i, W in [(1, W1), (2, W2)]:
        D = np.zeros((8, 8), dtype=np.float32)
        for k in range(8):
            for j in range(4):
                D[k, j] = W[WMAP[k], j]
                D[k, j + 4] = W[WMAP[k], j]
        t[f"lD{i}"] = _blockdiag(D, 16)
    PO = np.zeros((8, 4), dtype=np.float32)
    for k in range(8):
        PO[k, WMAP[k]] = 1.0
    t["lPO"] = _blockdiag(PO, 16)
    consts = np.zeros((128, 4), dtype=np.float32)
    for i, b in enumerate((b0, b1, b2)):
        consts[:, i] = np.tile(np.tile(np.asarray(b, np.float32), 2), 16)
    consts[:, 3] = np.tile([0., 0., 0., 0., PI2, PI2, PI2, PI2], 16)
    t["consts"] = consts
    return t


# ---------------- device kernel ----------------
MASK_A = [0, 0, 4, 4, 0, 0, 0, 0]
MASK_B = [3, 7, 3, 7, 0, 0, 0, 0]
W_NAMES = ["lD0", "lD1", "lD2", "lA1_0", "lA2_0", "lA1_1", "lA2_1",
           "lA1_2", "lA2_2", "lPO", "consts"]
W_COLS = {"lD0": 64, "lD1": 128, "lD2": 128, "lA1_0": 128, "lA2_0": 128,
          "lA1_1": 128, "lA2_1": 128, "lA1_2": 128, "lA2_2": 128,
          "lPO": 64, "consts": 4}


def build_kernel(tc, x, out, wins):
    nc = tc.nc
    n_st = B_CORE // 8192
    shufA = [8 * t_ + MASK_A[j] for t_ in range(4) for j in range(8)]
    shufB = [8 * t_ + MASK_B[j] for t_ in range(4) for j in range(8)]
    with contextlib.ExitStack() as ctx:
        wpool = ctx.enter_context(tc.tile_pool(name="w", bufs=1))
        slabs = ctx.enter_context(tc.tile_pool(name="slabs", bufs=8))
        sb = ctx.enter_context(tc.tile_pool(name="sb", bufs=2))
        ps_xk = ctx.enter_context(tc.tile_pool(name="ps_xk", bufs=1, space="PSUM"))
        ps_mm = ctx.enter_context(tc.tile_pool(name="ps_mm", bufs=2, space="PSUM"))
        ps_o = ctx.enter_context(tc.tile_pool(name="ps_o", bufs=1, space="PSUM"))

        wt = {}
        for name in W_NAMES:
            wtile = wpool.tile([128, W_COLS[name]], F32, tag=name)
            nc.sync.dma_start(wtile[:], wins[name][:, :])
            wt[name] = wtile
        ctile = wt["consts"]
        ident = wpool.tile([128, 128], F32, tag="ident")
        make_identity(nc, ident)
        lA1 = [wt["lA1_0"], wt["lA1_1"], wt["lA1_2"]]
        lA2 = [wt["lA2_0"], wt["lA2_1"], wt["lA2_2"]]
        lD = [wt["lD0"], wt["lD1"], wt["lD2"]]

        xv = x.rearrange("(st s p r) f -> st s p (r f)", st=n_st, s=4, p=128)
        ov = out.rearrange("(st s p r) w -> st s p (r w)", st=n_st, s=4, p=128)

        for st in [i % n_st for i in range(BENCH_REPS * n_st)]:
            xk = ps_xk.tile([128, 2, 512], F32, tag="xk")
            for s in range(4):
                slab = slabs.tile([128, 256], F32, tag="slab")
                nc.sync.dma_start(slab[:], xv[st, s])
                nc.tensor.transpose(xk[:, 0, s * 128:(s + 1) * 128], slab[:, 0:128], ident[:])
                nc.tensor.transpose(xk[:, 1, s * 128:(s + 1) * 128], slab[:, 128:256], ident[:])
            sxk = sb.tile([128, 2, 512], F32, tag="sxk")
            nc.vector.tensor_copy(sxk[:, 0], xk[:, 0])
            nc.scalar.copy(sxk[:, 1], xk[:, 1])

            vin = None
            for li in range(3):
                pre = ps_mm.tile([128, 512], F32, tag="pre")
                if li == 0:
                    nc.tensor.matmul(pre[0:64, :], wt["lD0"][:], sxk[:, 0], start=True, stop=True)
                    nc.tensor.matmul(pre[64:128, :], wt["lD0"][:], sxk[:, 1], start=True, stop=True)
                else:
                    nc.tensor.matmul(pre[:, :], lD[li][:], vin[:], start=True, stop=True)
                h8 = sb.tile([128, 512], F32, tag="h8")
                nc.scalar.activation(h8[:], pre[:], mybir.ActivationFunctionType.Tanh,
                                     bias=ctile[:, li:li + 1], scale=1.0)
                trig = sb.tile([128, 512], F32, tag="trig")
                nc.scalar.activation(trig[:], h8[:], mybir.ActivationFunctionType.Sin,
                                     bias=ctile[:, 3:4], scale=1.0)
                ga = sb.tile([128, 512], F32, tag="ga")
                gb = sb.tile([128, 512], F32, tag="gb")
                nc.vector.stream_shuffle(ga[:], trig[:], shufA)
                nc.vector.stream_shuffle(gb[:], trig[:], shufB)
                g = sb.tile([128, 512], F32, tag="g")
                nc.vector.tensor_mul(g[:], ga[:], gb[:])
                r1 = ps_mm.tile([128, 512], F32, tag="r1")
                nc.tensor.matmul(r1[:], lA1[li][:], g[:], start=True, stop=False)
                nc.tensor.matmul(r1[:], lA2[li][:], trig[:], start=False, stop=True)
                v = sb.tile([128, 512], F32, tag="v")
                nc.vector.tensor_mul(v[:], trig[:], r1[:])
                vin = v

            po = ps_o.tile([64, 512], F32, tag="po")
            nc.tensor.matmul(po[:, :], wt["lPO"][:], vin[:], start=True, stop=True)
            so = sb.tile([64, 512], F32, tag="so")
            nc.scalar.copy(so[:], po[:])
            ob = ps_o.tile([128, 256], F32, tag="ob")
            for s in range(4):
                nc.tensor.transpose(ob[:, s * 64:(s + 1) * 64], so[:, s * 128:(s + 1) * 128],
                                    ident[0:64, 0:64])
            sob = sb.tile([128, 256], F32, tag="sob")
            nc.vector.tensor_copy(sob[:], ob[:])
            for s in range(4):
                nc.sync.dma_start(ov[st, s], sob[:, s * 64:(s + 1) * 64])



# Force Tanh/Sin into a single resident ACT table set (silu_and_others holds
# both) so the table-load pass doesn't thrash between per-func sets. Dict
# order/indices are preserved so act_func_set_id stays consistent.
from concourse import hw_specs as _hw_specs
import concourse.bacc as _bacc_mod
_orig_get_tables = _hw_specs.get_activation_tables

def _patched_get_tables(arch):
    tabs = _orig_get_tables(arch)
    out = {}
    for name, s in tabs.items():
        s2 = set(s)
        if name != "silu_and_others":
            s2.discard(mybir.ActivationFunctionType.Tanh)
            s2.discard(mybir.ActivationFunctionType.Sin)
        out[name] = s2
    return out

_hw_specs.get_activation_tables = _patched_get_tables
for _mod in (_bacc_mod,):
    if hasattr(_mod, "get_activation_tables"):
        _mod.get_activation_tables = _patched_get_tables


_CACHE = {}


def _get_compiled():
    if "nc" in _CACHE:
        return _CACHE["nc"], _CACHE["tiles"]
    nc = bacc.Bacc("TRN2", target_bir_lowering=False, debug=False,
                   num_devices=N_CORES)
    x_ap = nc.dram_tensor("x", [B_CORE, D_IN], F32, kind="ExternalInput").ap()
    out_ap = nc.dram_tensor("out", [B_CORE, 4], F32, kind="ExternalOutput").ap()
    wins = {}
    for name in W_NAMES:
        wins[name] = nc.dram_tensor(name, [128, W_COLS[name]], F32,
                                    kind="ExternalInput").ap()
    with tile.TileContext(nc) as tc:
        build_kernel(tc, x_ap, out_ap, wins)
    nc.compile()
    _CACHE["nc"] = nc
    _CACHE["tiles"] = None
    return nc, None


def kernel(x, theta, W0, b0, W1, b1, W2, b2):
    x = np.ascontiguousarray(np.asarray(x, dtype=np.float32))
    wt = host_tensors(np.asarray(theta), np.asarray(W0), np.asarray(b0),
                      np.asarray(W1), np.asarray(b1), np.asarray(W2),
                      np.asarray(b2))
    nc, _ = _get_compiled()
    in_maps = []
    for c in range(N_CORES):
        m = {"x": np.ascontiguousarray(x[c * B_CORE:(c + 1) * B_CORE])}
        for name in W_NAMES:
            m[name] = wt[name] if name != "consts" else wt["consts"]
        in_maps.append(m)
    res = run_bass_kernel_spmd(nc, in_maps, core_ids=list(range(N_CORES)))
    outs = [res.results[c]["out"] for c in range(N_CORES)]
    return np.concatenate(outs, axis=0).astype(np.float32)



# revision 6
# speedup vs baseline: 1.4792x; 1.4792x over previous
"""Self-contained Trainium2 Bass kernel for the HQNN problem.

Math: the 4-qubit circuit after angle embedding applies a fixed unitary whose
Heisenberg-evolved Z observables are sparse Pauli sums over {I,Y,Z}; each
hybrid layer reduces to tanh -> sin/cos -> a few elementwise products -> small
static matmuls (folded with the next Dense layer). Data-parallel over 8 cores.

V2: fp16 compute, host-side pre-transpose of x into the on-chip layout
(partition = (r, feature), columns = batch) so the kernel does no PE
transposes, and large [128,1024] tiles to amortize per-op overheads.
"""
import os
import sys
sys.path.insert(0, "/opt/trn_rl_repo")
import itertools
import contextlib
import numpy as np

import concourse.bass as bass
import concourse.bacc as bacc
import concourse.tile as tile
from concourse import mybir
from concourse.bass_utils import run_bass_kernel_spmd

F32 = mybir.dt.float32
F16 = mybir.dt.float16
NPF16 = np.float16
PI2 = float(np.pi / 2)
N_CORES = 8
B_TOTAL, D_IN = 524288, 16
B_CORE = B_TOTAL // N_CORES

# tile geometry: C columns per tile, 16 groups x 8 slots in partitions,
# each super-tile covers 16*C rows (2 halves x 8 r-groups x C columns)
C = 1024
ROWS_ST = 16 * C
N_ST = B_CORE // ROWS_ST

# bench-only: repeat the kernel body N times so on-HW body time can be
# measured as a slope over reps (dispatch overhead dominates single calls).
BENCH_REPS = int(os.environ.get("BASS_BENCH_REPS", "1"))

# ---------------- host-side math ----------------
_I2 = np.eye(2, dtype=complex)
_PY = np.array([[0, -1j], [1j, 0]])
_PZ = np.array([[1, 0], [0, -1]], dtype=complex)
SUPPORTS = [(0, 1, 3), (0, 2, 3), (1, 3), (0, 2)]


def _kron(ms):
    out = np.array([[1.0 + 0j]])
    for m in ms:
        out = np.kron(out, m)
    return out


def _op_on(w, m):
    return _kron([m if v == w else _I2 for v in range(4)])


def _layer_tensors(theta_l):
    U = np.eye(16, dtype=complex)
    for l in range(2):
        for w in range(4):
            c, s = np.cos(theta_l[l, w] / 2), np.sin(theta_l[l, w] / 2)
            U = _op_on(w, np.array([[c, -1j * s], [-1j * s, c]])) @ U
        for w in range(4):
            t = (w + 1) % 4
            Cm = np.zeros((16, 16), dtype=complex)
            for k in range(16):
                bits = [(k >> (3 - v)) & 1 for v in range(4)]
                if bits[w] == 1:
                    bits[t] ^= 1
                Cm[sum(b << (3 - v) for v, b in enumerate(bits)), k] = 1
            U = Cm @ U
    letters = {"I": _I2, "Y": _PY, "Z": _PZ}
    out = []
    for w, sup in enumerate(SUPPORTS):
        H = U.conj().T @ _op_on(w, _PZ) @ U
        T = np.zeros((2,) * len(sup))
        for s in itertools.product("IYZ", repeat=4):
            P = _kron([letters[c] for c in s])
            co = float(np.real(np.trace(P.conj().T @ H) / 16))
            if abs(co) < 1e-10:
                continue
            nz = tuple(v for v in range(4) if s[v] != "I")
            assert set(nz).issubset(set(sup)), f"support {s} w={w}"
            idx, sign = [], 1.0
            ok = True
            for v in sup:
                if s[v] == "I":
                    ok = False
                    break
                idx.append(0 if s[v] == "Y" else 1)
                if s[v] == "Y":
                    sign = -sign
            if not ok:
                assert abs(co) < 1e-10
                continue
            T[tuple(idx)] = sign * co
        out.append(T)
    return out  # C0, C1, B2, B3


def _blockdiag(blk, n):
    K, M = blk.shape
    out = np.zeros((K * n, M * n), dtype=np.float32)
    for i in range(n):
        out[i * K:(i + 1) * K, i * M:(i + 1) * M] = blk
    return out


WMAP = [3, 0, 1, 2, 3, 0, 1, 2]


def host_tensors(theta, W0, b0, W1, b1, W2, b2):
    t = {}
    for i in range(3):
        C0, C1, B2, B3 = _layer_tensors(np.asarray(theta[i], dtype=np.float64))
        A1 = np.zeros((8, 8), dtype=np.float32)
        for a in range(2):
            for c in range(2):
                gi = a * 2 + c
                A1[gi, 1] = C0[a, 0, c]
                A1[gi, 5] = C0[a, 1, c]
                A1[gi, 2] = C1[a, 0, c]
                A1[gi, 6] = C1[a, 1, c]
        A2 = np.zeros((8, 8), dtype=np.float32)
        for b in range(2):
            A2[1 + 4 * b, 3] = B2[b, 0]
            A2[1 + 4 * b, 7] = B2[b, 1]
            A2[2 + 4 * b, 0] = B3[0, b]
            A2[2 + 4 * b, 4] = B3[1, b]
        t[f"lA1_{i}"] = _blockdiag(A1, 16).astype(NPF16)
        t[f"lA2_{i}"] = _blockdiag(A2, 16).astype(NPF16)
    D0 = np.zeros((16, 8), dtype=np.float32)
    D0[:, 0:4] = W0
    D0[:, 4:8] = W0
    t["lD0"] = _blockdiag(D0, 8).astype(NPF16)
    for i, W in [(1, W1), (2, W2)]:
        D = np.zeros((8, 8), dtype=np.float32)
        for k in range(8):
            for j in range(4):
                D[k, j] = W[WMAP[k], j]
                D[k, j + 4] = W[WMAP[k], j]
        t[f"lD{i}"] = _blockdiag(D, 16).astype(NPF16)
    PO = np.zeros((8, 4), dtype=np.float32)
    for k in range(8):
        PO[k, WMAP[k]] = 1.0
    t["lPO"] = _blockdiag(PO, 16).astype(NPF16)
    consts = np.zeros((128, 4), dtype=np.float32)
    for i, b in enumerate((b0, b1, b2)):
        consts[:, i] = np.tile(np.tile(np.asarray(b, np.float32), 2), 16)
    consts[:, 3] = np.tile([0., 0., 0., 0., PI2, PI2, PI2, PI2], 16)
    t["consts"] = consts
    return t


# ---------------- device kernel ----------------
MASK_A = [0, 0, 4, 4, 0, 0, 0, 0]
MASK_B = [3, 7, 3, 7, 0, 0, 0, 0]
W_NAMES = ["lD0", "lD1", "lD2", "lA1_0", "lA2_0", "lA1_1", "lA2_1",
           "lA1_2", "lA2_2", "lPO", "consts"]
W_COLS = {"lD0": 64, "lD1": 128, "lD2": 128, "lA1_0": 128, "lA2_0": 128,
          "lA1_1": 128, "lA2_1": 128, "lA1_2": 128, "lA2_2": 128,
          "lPO": 64, "consts": 4}
W_DT = {n: (F32 if n == "consts" else F16) for n in W_NAMES}


def build_kernel(tc, xt, out, wins):
    nc = tc.nc
    shufA = [8 * t_ + MASK_A[j] for t_ in range(4) for j in range(8)]
    shufB = [8 * t_ + MASK_B[j] for t_ in range(4) for j in range(8)]
    with contextlib.ExitStack() as ctx:
        wpool = ctx.enter_context(tc.tile_pool(name="w", bufs=1))
        sxp = ctx.enter_context(tc.tile_pool(name="sx", bufs=2))
        sb = ctx.enter_context(tc.tile_pool(name="sb", bufs=2))
        ps_pre = ctx.enter_context(tc.tile_pool(name="ps_pre", bufs=2, space="PSUM"))
        ps_r1 = ctx.enter_context(tc.tile_pool(name="ps_r1", bufs=1, space="PSUM"))
        ps_po = ctx.enter_context(tc.tile_pool(name="ps_po", bufs=1, space="PSUM"))

        wt = {}
        for name in W_NAMES:
            wtile = wpool.tile([128, W_COLS[name]], W_DT[name], tag=name)
            nc.sync.dma_start(wtile[:], wins[name][:, :])
            wt[name] = wtile
        ctile = wt["consts"]
        lA1 = [wt["lA1_0"], wt["lA1_1"], wt["lA1_2"]]
        lA2 = [wt["lA2_0"], wt["lA2_1"], wt["lA2_2"]]
        lD = [wt["lD0"], wt["lD1"], wt["lD2"]]
        NCH = C // 512  # 512-col matmul chunks per tile

        for st in [i % N_ST for i in range(BENCH_REPS * N_ST)]:
            sx = sxp.tile([128, 2 * C], F16, tag="sx")
            nc.sync.dma_start(sx[:], xt[:, st * 2 * C:(st + 1) * 2 * C])

            vin = None
            for li in range(3):
                pre = ps_pre.tile([128, C], F32, tag="pre")
                if li == 0:
                    for ch in range(NCH):
                        a, b = ch * 512, (ch + 1) * 512
                        nc.tensor.matmul(pre[0:64, a:b], wt["lD0"][:],
                                         sx[:, a:b], start=True, stop=True)
                        nc.tensor.matmul(pre[64:128, a:b], wt["lD0"][:],
                                         sx[:, C + a:C + b], start=True, stop=True)
                else:
                    for ch in range(NCH):
                        a, b = ch * 512, (ch + 1) * 512
                        nc.tensor.matmul(pre[:, a:b], lD[li][:], vin[:, a:b],
                                         start=True, stop=True)
                h8 = sb.tile([128, C], F16, tag="h8")
                nc.scalar.activation(h8[:], pre[:], mybir.ActivationFunctionType.Tanh,
                                     bias=ctile[:, li:li + 1], scale=1.0)
                trig = sb.tile([128, C], F16, tag="trig")
                nc.scalar.activation(trig[:], h8[:], mybir.ActivationFunctionType.Sin,
                                     bias=ctile[:, 3:4], scale=1.0)
                ga = sb.tile([128, C], F16, tag="ga")
                gb = sb.tile([128, C], F16, tag="gb")
                nc.vector.stream_shuffle(ga[:], trig[:], shufA)
                nc.vector.stream_shuffle(gb[:], trig[:], shufB)
                g = sb.tile([128, C], F16, tag="g")
                nc.gpsimd.tensor_mul(g[:], ga[:], gb[:])
                r1 = ps_r1.tile([128, C], F32, tag="r1")
                for ch in range(NCH):
                    a, b = ch * 512, (ch + 1) * 512
                    nc.tensor.matmul(r1[:, a:b], lA1[li][:], g[:, a:b],
                                     start=True, stop=False)
                    nc.tensor.matmul(r1[:, a:b], lA2[li][:], trig[:, a:b],
                                     start=False, stop=True)
                v = sb.tile([128, C], F16, tag="v")
                nc.vector.tensor_mul(v[:], trig[:], r1[:])
                vin = v

            po = ps_po.tile([64, C], F32, tag="po")
            for ch in range(NCH):
                a, b = ch * 512, (ch + 1) * 512
                nc.tensor.matmul(po[:, a:b], wt["lPO"][:], vin[:, a:b],
                                 start=True, stop=True)
            so = sb.tile([64, C], F16, tag="so")
            nc.scalar.copy(so[:], po[:])
            nc.sync.dma_start(out[:, st * C:(st + 1) * C], so[:])


# Force Tanh/Sin into a single resident ACT table set (silu_and_others holds
# both) so the table-load pass doesn't thrash between per-func sets. Dict
# order/indices are preserved so act_func_set_id stays consistent.
from concourse import hw_specs as _hw_specs
import concourse.bacc as _bacc_mod
_orig_get_tables = _hw_specs.get_activation_tables

def _patched_get_tables(arch):
    tabs = _orig_get_tables(arch)
    out = {}
    for name, s in tabs.items():
        s2 = set(s)
        if name != "silu_and_others":
            s2.discard(mybir.ActivationFunctionType.Tanh)
            s2.discard(mybir.ActivationFunctionType.Sin)
        out[name] = s2
    return out

_hw_specs.get_activation_tables = _patched_get_tables
for _mod in (_bacc_mod,):
    if hasattr(_mod, "get_activation_tables"):
        _mod.get_activation_tables = _patched_get_tables


_CACHE = {}


def _get_compiled():
    if "nc" in _CACHE:
        return _CACHE["nc"], _CACHE["tiles"]
    nc = bacc.Bacc("TRN2", target_bir_lowering=False, debug=False,
                   num_devices=N_CORES)
    xt_ap = nc.dram_tensor("xt", [128, N_ST * 2 * C], F16, kind="ExternalInput").ap()
    out_ap = nc.dram_tensor("out", [64, N_ST * C], F16, kind="ExternalOutput").ap()
    wins = {}
    for name in W_NAMES:
        wins[name] = nc.dram_tensor(name, [128, W_COLS[name]], W_DT[name],
                                    kind="ExternalInput").ap()
    with tile.TileContext(nc) as tc:
        build_kernel(tc, xt_ap, out_ap, wins)
    nc.compile()
    _CACHE["nc"] = nc
    _CACHE["tiles"] = None
    return nc, None


def host_pack_x(x):
    """x [B_TOTAL,16] f32 -> per-core [128, N_ST*2C] f16 in (r,f) x (st,half,c)
    layout."""
    xc = x.reshape(N_CORES, N_ST, 2, 8, C, D_IN)     # (core, st, half, r, c, f)
    xt = xc.transpose(0, 3, 5, 1, 2, 4)               # (core, r, f, st, half, c)
    return np.ascontiguousarray(xt.reshape(N_CORES, 128, N_ST * 2 * C)
                                .astype(NPF16))


def host_unpack_out(res_outs):
    """per-core [64, N_ST*C] f16 (partition=(half,r,w), col=(st,c)) ->
    [B_TOTAL, 4] f32."""
    o = np.stack(res_outs, axis=0).reshape(N_CORES, 2, 8, 4, N_ST, C)
    # (core, half, r, w, st, c) -> (core, st, half, r, c, w)
    o = o.transpose(0, 4, 1, 2, 5, 3)
    return np.ascontiguousarray(o.reshape(B_TOTAL, 4).astype(np.float32))


def kernel(x, theta, W0, b0, W1, b1, W2, b2):
    x = np.asarray(x, dtype=np.float32)
    wt = host_tensors(np.asarray(theta), np.asarray(W0), np.asarray(b0),
                      np.asarray(W1), np.asarray(b1), np.asarray(W2),
                      np.asarray(b2))
    nc, _ = _get_compiled()
    xt = host_pack_x(x)
    in_maps = []
    for c in range(N_CORES):
        m = {"xt": xt[c]}
        for name in W_NAMES:
            m[name] = wt[name]
        in_maps.append(m)
    res = run_bass_kernel_spmd(nc, in_maps, core_ids=list(range(N_CORES)))
    outs = [res.results[c]["out"] for c in range(N_CORES)]
    return host_unpack_out(outs)


# revision 15
# speedup vs baseline: 2.0426x; 1.3808x over previous
"""Self-contained Trainium2 Bass kernel for the HQNN problem.

Math: the 4-qubit circuit after angle embedding applies a fixed unitary whose
Heisenberg-evolved Z observables are sparse Pauli sums over {I,Y,Z}; each
hybrid layer reduces to tanh -> sin/cos -> a few elementwise products -> small
static matmuls (folded with the next Dense layer). Data-parallel over 8 cores.

V2: fp16 compute, host-side pre-transpose of x into the on-chip layout
(partition = (r, feature), columns = batch) so the kernel does no PE
transposes, and large [128,1024] tiles to amortize per-op overheads.
"""
import os
import sys
sys.path.insert(0, "/opt/trn_rl_repo")
import itertools
import contextlib
import numpy as np

import concourse.bass as bass
import concourse.bacc as bacc
import concourse.tile as tile
from concourse import mybir
from concourse.bass_utils import run_bass_kernel_spmd

F32 = mybir.dt.float32
F16 = mybir.dt.float16
NPF16 = np.float16
PI2 = float(np.pi / 2)
N_CORES = 8
B_TOTAL, D_IN = 524288, 16
B_CORE = B_TOTAL // N_CORES

# tile geometry: C columns per tile, 16 groups x 8 slots in partitions,
# each super-tile covers 16*C rows (2 halves x 8 r-groups x C columns)
C = 1024
ROWS_ST = 16 * C
N_ST = B_CORE // ROWS_ST

# bench-only: repeat the kernel body N times so on-HW body time can be
# measured as a slope over reps (dispatch overhead dominates single calls).
BENCH_REPS = int(os.environ.get("BASS_BENCH_REPS", "1"))

# ---------------- host-side math ----------------
_I2 = np.eye(2, dtype=complex)
_PY = np.array([[0, -1j], [1j, 0]])
_PZ = np.array([[1, 0], [0, -1]], dtype=complex)
SUPPORTS = [(0, 1, 3), (0, 2, 3), (1, 3), (0, 2)]


def _kron(ms):
    out = np.array([[1.0 + 0j]])
    for m in ms:
        out = np.kron(out, m)
    return out


def _op_on(w, m):
    return _kron([m if v == w else _I2 for v in range(4)])


def _layer_tensors(theta_l):
    U = np.eye(16, dtype=complex)
    for l in range(2):
        for w in range(4):
            c, s = np.cos(theta_l[l, w] / 2), np.sin(theta_l[l, w] / 2)
            U = _op_on(w, np.array([[c, -1j * s], [-1j * s, c]])) @ U
        for w in range(4):
            t = (w + 1) % 4
            Cm = np.zeros((16, 16), dtype=complex)
            for k in range(16):
                bits = [(k >> (3 - v)) & 1 for v in range(4)]
                if bits[w] == 1:
                    bits[t] ^= 1
                Cm[sum(b << (3 - v) for v, b in enumerate(bits)), k] = 1
            U = Cm @ U
    letters = {"I": _I2, "Y": _PY, "Z": _PZ}
    out = []
    for w, sup in enumerate(SUPPORTS):
        H = U.conj().T @ _op_on(w, _PZ) @ U
        T = np.zeros((2,) * len(sup))
        for s in itertools.product("IYZ", repeat=4):
            P = _kron([letters[c] for c in s])
            co = float(np.real(np.trace(P.conj().T @ H) / 16))
            if abs(co) < 1e-10:
                continue
            nz = tuple(v for v in range(4) if s[v] != "I")
            assert set(nz).issubset(set(sup)), f"support {s} w={w}"
            idx, sign = [], 1.0
            ok = True
            for v in sup:
                if s[v] == "I":
                    ok = False
                    break
                idx.append(0 if s[v] == "Y" else 1)
                if s[v] == "Y":
                    sign = -sign
            if not ok:
                assert abs(co) < 1e-10
                continue
            T[tuple(idx)] = sign * co
        out.append(T)
    return out  # C0, C1, B2, B3


def _blockdiag(blk, n):
    K, M = blk.shape
    out = np.zeros((K * n, M * n), dtype=np.float32)
    for i in range(n):
        out[i * K:(i + 1) * K, i * M:(i + 1) * M] = blk
    return out


WMAP = [3, 0, 1, 2, 3, 0, 1, 2]


def host_tensors(theta, W0, b0, W1, b1, W2, b2):
    t = {}
    for i in range(3):
        C0, C1, B2, B3 = _layer_tensors(np.asarray(theta[i], dtype=np.float64))
        A1 = np.zeros((8, 8), dtype=np.float32)
        for a in range(2):
            for c in range(2):
                gi = a * 2 + c
                A1[gi, 1] = C0[a, 0, c]
                A1[gi, 5] = C0[a, 1, c]
                A1[gi, 2] = C1[a, 0, c]
                A1[gi, 6] = C1[a, 1, c]
        A2 = np.zeros((8, 8), dtype=np.float32)
        for b in range(2):
            A2[1 + 4 * b, 3] = B2[b, 0]
            A2[1 + 4 * b, 7] = B2[b, 1]
            A2[2 + 4 * b, 0] = B3[0, b]
            A2[2 + 4 * b, 4] = B3[1, b]
        # remap A1 rows for the DMA-gather g layout: ga=[s0 x4, c0 x4],
        # gb=[s3,c3]x4 -> product (a,c) lands in g slot 4a+c (and a dup at
        # 4a+2+c); read each product once via rows {0,1,4,5}.
        A1n = np.zeros((8, 8), dtype=np.float32)
        A1n[[0, 1, 4, 5], :] = A1[[0, 1, 2, 3], :]
        t[f"lA1_{i}"] = _blockdiag(A1n, 16).astype(NPF16)
        t[f"lA2_{i}"] = _blockdiag(A2, 16).astype(NPF16)
    D0 = np.zeros((16, 8), dtype=np.float32)
    D0[:, 0:4] = W0
    D0[:, 4:8] = W0
    t["lD0"] = _blockdiag(D0, 8).astype(NPF16)
    for i, W in [(1, W1), (2, W2)]:
        D = np.zeros((8, 8), dtype=np.float32)
        for k in range(8):
            for j in range(4):
                D[k, j] = W[WMAP[k], j]
                D[k, j + 4] = W[WMAP[k], j]
        t[f"lD{i}"] = _blockdiag(D, 16).astype(NPF16)
    PO = np.zeros((8, 4), dtype=np.float32)
    for k in range(8):
        PO[k, WMAP[k]] = 1.0
    t["lPO"] = _blockdiag(PO, 16).astype(NPF16)
    consts = np.zeros((128, 4), dtype=np.float32)
    for i, b in enumerate((b0, b1, b2)):
        consts[:, i] = np.tile(np.tile(np.asarray(b, np.float32), 2), 16)
    consts[:, 3] = np.tile([0., 0., 0., 0., PI2, PI2, PI2, PI2], 16)
    t["consts"] = consts
    return t


# ---------------- device kernel ----------------
MASK_A = [0, 0, 4, 4, 0, 0, 0, 0]
MASK_B = [3, 7, 3, 7, 0, 0, 0, 0]
W_NAMES = ["lD0", "lD1", "lD2", "lA1_0", "lA2_0", "lA1_1", "lA2_1",
           "lA1_2", "lA2_2", "lPO", "consts"]
W_COLS = {"lD0": 64, "lD1": 128, "lD2": 128, "lA1_0": 128, "lA2_0": 128,
          "lA1_1": 128, "lA2_1": 128, "lA1_2": 128, "lA2_2": 128,
          "lPO": 64, "consts": 4}
W_DT = {n: (F32 if n == "consts" else F16) for n in W_NAMES}


def build_kernel(tc, xt, out, wins):
    nc = tc.nc
    with contextlib.ExitStack() as ctx:
        wpool = ctx.enter_context(tc.tile_pool(name="w", bufs=1))
        sxp = ctx.enter_context(tc.tile_pool(name="sx", bufs=2))
        sb = ctx.enter_context(tc.tile_pool(name="sb", bufs=4))
        ps_pre = ctx.enter_context(tc.tile_pool(name="ps_pre", bufs=2, space="PSUM"))
        ps_r1 = ctx.enter_context(tc.tile_pool(name="ps_r1", bufs=1, space="PSUM"))
        ps_po = ctx.enter_context(tc.tile_pool(name="ps_po", bufs=1, space="PSUM"))

        wt = {}
        for name in W_NAMES:
            wtile = wpool.tile([128, W_COLS[name]], W_DT[name], tag=name)
            nc.sync.dma_start(wtile[:], wins[name][:, :])
            wt[name] = wtile
        ctile = wt["consts"]
        lA1 = [wt["lA1_0"], wt["lA1_1"], wt["lA1_2"]]
        lA2 = [wt["lA2_0"], wt["lA2_1"], wt["lA2_2"]]
        lD = [wt["lD0"], wt["lD1"], wt["lD2"]]

        # full-coverage shuffle masks matching the A1n row remap:
        # ga = [s0 x4, c0 x4], gb = [s3,c3]x4 per 8-slot group
        shufA = [8 * t_ + (0 if j < 4 else 4) for t_ in range(4) for j in range(8)]
        shufB = [8 * t_ + (3 if j % 2 == 0 else 7) for t_ in range(4) for j in range(8)]

        vins = {}

        def do_layer(st, li, sx):
            vin = vins.get(st)
            pre = ps_pre.tile([128, C], F32, tag="pre")
            if li == 0:
                for a in range(0, C, 512):
                    nc.tensor.matmul(pre[0:64, a:a + 512], wt["lD0"][:],
                                     sx[:, a:a + 512], start=True, stop=True)
                    nc.tensor.matmul(pre[64:128, a:a + 512], wt["lD0"][:],
                                     sx[:, C + a:C + a + 512],
                                     start=True, stop=True)
            else:
                for a in range(0, C, 512):
                    nc.tensor.matmul(pre[:, a:a + 512], lD[li][:],
                                     vin[:, a:a + 512], start=True, stop=True)
            h8 = sb.tile([128, C], F16, tag="h8")
            nc.scalar.activation(h8[:], pre[:], mybir.ActivationFunctionType.Tanh,
                                 bias=ctile[:, li:li + 1], scale=1.0)
            trig = sb.tile([128, C], F16, tag="trig")
            nc.scalar.activation(trig[:], h8[:], mybir.ActivationFunctionType.Sin,
                                 bias=ctile[:, 3:4], scale=1.0)
            # partition-permutes run as f32-pair views: halves DVE FD
            ga = sb.tile([128, C], F16, tag="ga")
            gb = sb.tile([128, C], F16, tag="gb")
            nc.vector.stream_shuffle(ga[:].bitcast(F32), trig[:].bitcast(F32),
                                     shufA)
            nc.vector.stream_shuffle(gb[:].bitcast(F32), trig[:].bitcast(F32),
                                     shufB)
            g = sb.tile([128, C], F16, tag="g")
            if li == 2:
                nc.vector.tensor_mul(g[:], ga[:], gb[:])
            else:
                nc.gpsimd.tensor_mul(g[:], ga[:], gb[:])
            r1 = ps_r1.tile([128, C], F32, tag="r1")
            for a in range(0, C, 512):
                nc.tensor.matmul(r1[:, a:a + 512], lA1[li][:], g[:, a:a + 512],
                                 start=True, stop=False)
                nc.tensor.matmul(r1[:, a:a + 512], lA2[li][:],
                                 trig[:, a:a + 512], start=False, stop=True)
            v = sb.tile([128, C], F16, tag="v")
            nc.vector.tensor_mul(v[:], trig[:], r1[:])
            vins[st] = v

        def do_po(key, st):
            vin = vins.pop(key)
            po = ps_po.tile([64, C], F32, tag="po")
            for a in range(0, C, 512):
                nc.tensor.matmul(po[:, a:a + 512], wt["lPO"][:],
                                 vin[:, a:a + 512], start=True, stop=True)
            so = sb.tile([64, C], F16, tag="so")
            nc.scalar.copy(so[:], po[:])
            nc.sync.dma_start(out[:, st * C:(st + 1) * C], so[:])

        # wavefront software pipeline: emit (st, stage) tasks diagonally so
        # the in-order per-engine queues interleave work from adjacent
        # super-tiles instead of serializing the long per-tile dep chain.
        sts = [i % N_ST for i in range(BENCH_REPS * N_ST)]
        for wave in range(len(sts) + 3):
            for i, st in enumerate(sts):
                stage = wave - i
                if stage < 0 or stage > 3:
                    continue
                if stage == 0:
                    sx = sxp.tile([128, 2 * C], F16, tag="sx")
                    nc.sync.dma_start(sx[:], xt[:, st * 2 * C:(st + 1) * 2 * C])
                    do_layer(i, 0, sx)
                elif stage in (1, 2):
                    do_layer(i, stage, None)
                else:
                    do_po(i, st)


# Force Tanh/Sin into a single resident ACT table set (silu_and_others holds
# both) so the table-load pass doesn't thrash between per-func sets. Dict
# order/indices are preserved so act_func_set_id stays consistent.
from concourse import hw_specs as _hw_specs
import concourse.bacc as _bacc_mod
_orig_get_tables = _hw_specs.get_activation_tables

def _patched_get_tables(arch):
    tabs = _orig_get_tables(arch)
    out = {}
    for name, s in tabs.items():
        s2 = set(s)
        if name != "silu_and_others":
            s2.discard(mybir.ActivationFunctionType.Tanh)
            s2.discard(mybir.ActivationFunctionType.Sin)
        out[name] = s2
    return out

_hw_specs.get_activation_tables = _patched_get_tables
for _mod in (_bacc_mod,):
    if hasattr(_mod, "get_activation_tables"):
        _mod.get_activation_tables = _patched_get_tables


_CACHE = {}


def _get_compiled():
    if "nc" in _CACHE:
        return _CACHE["nc"], _CACHE["tiles"]
    nc = bacc.Bacc("TRN2", target_bir_lowering=False, debug=False,
                   num_devices=N_CORES)
    xt_ap = nc.dram_tensor("xt", [128, N_ST * 2 * C], F16, kind="ExternalInput").ap()
    out_ap = nc.dram_tensor("out", [64, N_ST * C], F16, kind="ExternalOutput").ap()
    wins = {}
    for name in W_NAMES:
        wins[name] = nc.dram_tensor(name, [128, W_COLS[name]], W_DT[name],
                                    kind="ExternalInput").ap()
    with tile.TileContext(nc) as tc:
        build_kernel(tc, xt_ap, out_ap, wins)
    nc.compile()
    _CACHE["nc"] = nc
    _CACHE["tiles"] = None
    return nc, None


def host_pack_x(x):
    """x [B_TOTAL,16] f32 -> per-core [128, N_ST*2C] f16 in (r,f) x (st,half,c)
    layout."""
    xc = x.reshape(N_CORES, N_ST, 2, 8, C, D_IN)     # (core, st, half, r, c, f)
    xt = xc.transpose(0, 3, 5, 1, 2, 4)               # (core, r, f, st, half, c)
    return np.ascontiguousarray(xt.reshape(N_CORES, 128, N_ST * 2 * C)
                                .astype(NPF16))


def host_unpack_out(res_outs):
    """per-core [64, N_ST*C] f16 (partition=(half,r,w), col=(st,c)) ->
    [B_TOTAL, 4] f32."""
    o = np.stack(res_outs, axis=0).reshape(N_CORES, 2, 8, 4, N_ST, C)
    # (core, half, r, w, st, c) -> (core, st, half, r, c, w)
    o = o.transpose(0, 4, 1, 2, 5, 3)
    return np.ascontiguousarray(o.reshape(B_TOTAL, 4).astype(np.float32))


def kernel(x, theta, W0, b0, W1, b1, W2, b2):
    x = np.asarray(x, dtype=np.float32)
    wt = host_tensors(np.asarray(theta), np.asarray(W0), np.asarray(b0),
                      np.asarray(W1), np.asarray(b1), np.asarray(W2),
                      np.asarray(b2))
    nc, _ = _get_compiled()
    xt = host_pack_x(x)
    in_maps = []
    for c in range(N_CORES):
        m = {"xt": xt[c]}
        for name in W_NAMES:
            m[name] = wt[name]
        in_maps.append(m)
    res = run_bass_kernel_spmd(nc, in_maps, core_ids=list(range(N_CORES)))
    outs = [res.results[c]["out"] for c in range(N_CORES)]
    return host_unpack_out(outs)
